# revision 5
# baseline (speedup 1.0000x reference)
"""KV-cache scatter update kernel for 8 Trainium2 NeuronCores.

Full-input contract: kernel(**inputs) takes the unsharded tensors, shards
along the kv-heads dim (H=8 -> 1 head per core), runs a Bass kernel that
(a) bulk-copies each core's K+V cache shard DRAM->DRAM and (b) scatters the
32 new (kv, layer, batch) rows at position_ids via indirect DMA, then
reassembles the full (2, L, B, H, MAX_LEN, D) output on host.
"""

import sys

sys.path.insert(0, "/opt/trn_rl_repo")

import numpy as np

L = 2          # layers
B = 8          # batch
H = 8          # kv heads == n_cores
MAX_LEN = 4096
D = 128
NCORES = 8
SLABS = 2 * L * B            # 32 (kv, layer, batch) slabs per core
ROWS = SLABS * MAX_LEN       # 131072 rows of D f32 per core (64 MiB)

NCHUNK = 4                   # bulk copy split into NCHUNK parallel DMAs
ENGINES = "gpsimd"           # which engines issue the bulk-copy DMAs

TRACE = False                # test.py flips this to profile
LAST_RESULT = None           # stash of BassKernelResults for test.py

_compiled = None


def build_nc(nchunk=NCHUNK, engines=ENGINES, reps=1, scatter=True):
    """Build the per-core Bass program.

    engines: "gpsimd" | "sync" | "sync+scalar" — who issues the bulk DMAs.
    reps: execute the whole body N times back-to-back (for benchmarking;
          semaphore targets keep counting upward so no reset is needed).
    scatter: False drops the staging+scatter (bulk copy only; bench-only).
    """
    from concourse import bass, mybir

    nc = bass.Bass()
    cache_in = nc.dram_tensor(
        "cache_in", [ROWS, D], mybir.dt.float32, kind="ExternalInput"
    )
    newkv = nc.dram_tensor("newkv", [SLABS, D], mybir.dt.float32, kind="ExternalInput")
    offs = nc.dram_tensor("offs", [SLABS, 1], mybir.dt.int32, kind="ExternalInput")
    out = nc.dram_tensor("out", [ROWS, D], mybir.dt.float32, kind="ExternalOutput")

    per_iter = (3 + nchunk) * 16  # sem increments per iteration

    if engines == "gpsimd":
        chunk_eng = ["gpsimd"] * nchunk
    elif engines == "sync":
        chunk_eng = ["sync"] * nchunk
    elif engines == "sync+scalar":
        chunk_eng = ["sync" if i % 2 == 0 else "scalar" for i in range(nchunk)]
    else:
        raise ValueError(engines)

    with (
        nc.sbuf_tensor("newkv_sb", [SLABS, D], mybir.dt.float32) as newkv_sb,
        nc.sbuf_tensor("offs_sb", [SLABS, 1], mybir.dt.int32) as offs_sb,
        nc.semaphore("dma_sem") as dma_sem,
        nc.Block() as block,
    ):
        rows_per = ROWS // nchunk
        chunks = [slice(i * rows_per, (i + 1) * rows_per) for i in range(nchunk)]

        if not scatter:
            per_iter = nchunk * 16

        @block.gpsimd
        def _(g):
            for r in range(reps):
                base = r * per_iter
                if scatter:
                    # Stage scatter payload + indices into SBUF (concurrent
                    # with the bulk copy).
                    g.dma_start(out=newkv_sb[:], in_=newkv[:]).then_inc(dma_sem, 16)
                    g.dma_start(out=offs_sb[:], in_=offs[:]).then_inc(dma_sem, 16)
                for ename, sl in zip(chunk_eng, chunks):
                    if ename == "gpsimd":
                        g.dma_start(out=out[sl, :], in_=cache_in[sl, :]).then_inc(
                            dma_sem, 16
                        )
                if scatter:
                    # Scatter must not race the bulk copy (it overwrites rows).
                    g.wait_ge(dma_sem, base + (2 + nchunk) * 16)
                    g.indirect_dma_start(
                        out=out[:],
                        out_offset=bass.IndirectOffsetOnAxis(
                            ap=offs_sb[:, :1], axis=0
                        ),
                        in_=newkv_sb[:],
                        in_offset=None,
                    ).then_inc(dma_sem, 16)
                g.wait_ge(dma_sem, base + per_iter)

        for other in ("sync", "scalar"):
            if other not in chunk_eng:
                continue

            def _make(other):
                def body(e):
                    for r in range(reps):
                        base = r * per_iter
                        if r > 0:
                            # WAW across reps: rep r's bulk copy must follow
                            # rep r-1's scatter into the same rows.
                            e.wait_ge(dma_sem, base)
                        for ename, sl in zip(chunk_eng, chunks):
                            if ename == other:
                                e.dma_start(
                                    out=out[sl, :], in_=cache_in[sl, :]
                                ).then_inc(dma_sem, 16)

                return body

            getattr(block, other)(_make(other))

    return nc


def make_in_maps(k, v, nk, nv, pos):
    """Shard full inputs into per-core input maps (one head per core)."""
    base = np.arange(SLABS, dtype=np.int64) * MAX_LEN
    offs_v = (base + np.tile(pos, 2 * L)).astype(np.int32).reshape(SLABS, 1)

    in_maps = []
    for h in range(H):
        cache = np.empty((2, L, B, MAX_LEN, D), dtype=np.float32)
        cache[0] = k[:, :, h]
        cache[1] = v[:, :, h]
        new = np.empty((2, L, B, D), dtype=np.float32)
        new[0] = nk[:, :, h, 0]
        new[1] = nv[:, :, h, 0]
        in_maps.append(
            {
                "cache_in": cache.reshape(ROWS, D),
                "newkv": new.reshape(SLABS, D),
                "offs": offs_v,
            }
        )
    return in_maps


def kernel(k_caches, v_caches, new_keys, new_values, position_ids):
    global _compiled, LAST_RESULT
    from concourse.bass_utils import run_bass_kernel_spmd

    k = np.asarray(k_caches, dtype=np.float32)
    v = np.asarray(v_caches, dtype=np.float32)
    nk = np.asarray(new_keys, dtype=np.float32)
    nv = np.asarray(new_values, dtype=np.float32)
    pos = np.asarray(position_ids).reshape(-1).astype(np.int64)  # (B,)

    in_maps = make_in_maps(k, v, nk, nv, pos)

    if _compiled is None:
        _compiled = build_nc()

    bkr = run_bass_kernel_spmd(_compiled, in_maps, list(range(NCORES)), trace=TRACE)
    LAST_RESULT = bkr
    res = bkr.results

    full = np.empty((2, L, B, H, MAX_LEN, D), dtype=np.float32)
    for h in range(H):
        full[:, :, :, h] = np.asarray(res[h]["out"]).reshape(2, L, B, MAX_LEN, D)
    return full
